# revision 9
# baseline (speedup 1.0000x reference)
"""BLiqNet (liquid-ODE MLP, single RK4 step) Trainium2 kernel — fp16 edition.

Math (reference, fp32):
    u  = x @ Wx.T + bx                  # [B, H=128]
    uu = u @ Wu.T + b_ode
    f(h) = -h + tanh(h @ Wh.T + uu)
    RK4 with dt=2.0 from h0=u:  g_i = tanh(pre_i), with
      pre1 = (Wh+Wu)@u            pre2 = Wu@u + Wh@g1
      pre3 = (Wh+Wu)@u + Wh@t     (t = g2 - g1)
      pre4 = (Wu-Wh)@u + 2Wh@g3 - 2Wh@t
      h_out = (u - g1 + 2 g2 + g4) / 3
    y = h_out @ Wout.T + bout           # [B, 256]

Strategy vs the fp32 v1 (338 us):
  * fp16 I/O: x is cast to fp16 on the host (DMA in halves: 32 MB/core),
    y is produced as fp16 on chip and cast back on the host (16 MB/core).
    DMA floor drops from ~281 us to ~134 us.  Accuracy ~8e-4 << 2e-2 gate.
  * All SBUF tensors and weights fp16 (PSUM stays fp32).  16-bit weights
    enable FWL fast weight load, cutting matmul cost 269 -> ~215 ns.
  * ACT is the new critical engine (tanh only exists there; ~618 ns/op at
    N=512): it runs exactly the 4 tanh + one of the two y bias-copies.
    DVE runs the u copy, t, the pair-wide (2-tile) q/r/hsum combos at 2x
    throughput, and the other y bias-copy.
  * Full psplit banking: every matmul group gets its own PSUM bank with a
    ~1-pipeline-period lifetime: pu,pa,pb,pc,p4 + py(2) = 7 of 8 banks.

Per 512-col tile: PE 14 MM (~3.0 us), ACT 5 ops (~3.1 us), DVE ~2.8 us,
DMA 1.5 MB fp16 (~2.1 us).  Expected ~3.2 us/tile * 64 tiles ~ 205 us.
"""

import sys

sys.path.insert(0, "/opt/trn_rl_repo")

import numpy as np

from contextlib import ExitStack

import concourse.bacc as bacc
import concourse.tile as tile
from concourse import bass_utils, mybir

def _ensure_axon_hooks():
    """bass_utils' trace path imports antenv.axon_hooks, which is missing in
    some images.  Provide it (with the ctypes NTFF hook when available) so a
    BASS_TRACE=1 run profiles instead of crashing."""
    import types
    if "antenv.axon_hooks" in sys.modules:
        return
    try:
        import antenv
        mod = types.ModuleType("antenv.axon_hooks")
        mod._hook = None
        mod.set_axon_ntff_profile_hook = lambda h: setattr(mod, "_hook", h)
        mod.get_axon_ntff_profile_hook = lambda: mod._hook
        sys.modules["antenv.axon_hooks"] = mod
        antenv.axon_hooks = mod
        try:
            if "/root/.axon_site" not in sys.path:
                sys.path.insert(0, "/root/.axon_site")
            from trn_agent_boot.trn_boot import _ntff_profile_via_ctypes
            hook = _ntff_profile_via_ctypes("/opt/axon/libaxon_pjrt.so")
            if hook is not None:
                mod.set_axon_ntff_profile_hook(hook)
        except Exception:
            pass
    except Exception:
        pass


_ensure_axon_hooks()

F32 = mybir.dt.float32
F16 = mybir.dt.float16
AF = mybir.ActivationFunctionType
ALU = mybir.AluOpType

B, D_IN, H, D_OUT = 262144, 512, 128, 256
N_CORES = 8
B_CORE = B // N_CORES

# weight-stack slot indices (stack is [128, NW, 128] fp16 in DRAM)
NW = 12
(WX0, WX1, WX2, WX3, SL1, SLWU, SLWH, SL4U, SLWH2, SLWH2N,
 SWO0, SWO1) = range(NW)
NB = 5  # bias-stack columns: b1, b2, b4, by0, by1
_NC_CACHE: dict = {}


def _prep_weights(Wx, bx, Wh, Wu, b_ode, Wout, bout):
    """Pack all matmul lhsT blocks into one [128, NW, 128] fp16 stack plus a
    [128, NB] fp32 bias stack.  lhsT convention: out = lhsT.T @ rhs, so for
    pre = M @ g the block is M.T."""
    f = np.float32
    Wx, bx, Wh, Wu = Wx.astype(f), bx.astype(f), Wh.astype(f), Wu.astype(f)
    b_ode, Wout, bout = b_ode.astype(f), Wout.astype(f), bout.astype(f)

    ws = np.zeros((128, NW, 128), dtype=f)
    WxT = Wx.T  # [512, H]
    for k in range(4):
        ws[:, WX0 + k, :] = WxT[k * 128:(k + 1) * 128, :]
    ws[:, SL1, :] = (Wh + Wu).T
    ws[:, SLWU, :] = Wu.T
    ws[:, SLWH, :] = Wh.T
    ws[:, SL4U, :] = (Wu - Wh).T
    ws[:, SLWH2, :] = (2.0 * Wh).T
    ws[:, SLWH2N, :] = (-2.0 * Wh).T
    WoT3 = (Wout / 3.0).T  # [128, 256]
    ws[:, SWO0, :] = WoT3[:, 0:128]
    ws[:, SWO1, :] = WoT3[:, 128:256]

    bs = np.zeros((128, NB), dtype=f)
    bs[:, 0] = (Wh + Wu) @ bx + b_ode        # b1 (also b3)
    bs[:, 1] = Wu @ bx + b_ode               # b2
    bs[:, 2] = (Wu - Wh) @ bx + b_ode        # b4
    by = (Wout @ bx) / 3.0 + bout            # [256]
    bs[:, 3] = by[0:128]
    bs[:, 4] = by[128:256]
    return ws.astype(np.float16), bs


def _build(b_core: int, n_tile: int = 512, variant: str = "v2"):
    """Build + compile the per-core Tile kernel (SPMD across cores)."""
    assert n_tile == 512
    nc = bacc.Bacc("TRN2", target_bir_lowering=False, debug=False)

    xT_d = nc.dram_tensor("xT", [D_IN, b_core], F16, kind="ExternalInput")
    ws_d = nc.dram_tensor("ws", [128, NW, 128], F16, kind="ExternalInput")
    bs_d = nc.dram_tensor("bs", [128, NB], F32, kind="ExternalInput")
    yT_d = nc.dram_tensor("yT", [D_OUT, b_core], F16, kind="ExternalOutput")

    xT_r = xT_d.rearrange("(k p) n -> p k n", p=128)  # [128, 4, b_core]
    yT_r = yT_d.rearrange("(h p) n -> p h n", p=128)  # [128, 2, b_core]

    n_tiles = b_core // n_tile
    assert n_tiles % 2 == 0

    with tile.TileContext(nc) as tc, ExitStack() as ctx:
        cpool = ctx.enter_context(tc.tile_pool(name="const", bufs=1))
        xpool = ctx.enter_context(tc.tile_pool(name="x", bufs=2))
        x1pool = ctx.enter_context(tc.tile_pool(name="x1", bufs=4))
        upool = ctx.enter_context(tc.tile_pool(name="u", bufs=4))
        g1pool = ctx.enter_context(tc.tile_pool(name="g1", bufs=4))
        g2pool = ctx.enter_context(tc.tile_pool(name="g2", bufs=3))
        g3pool = ctx.enter_context(tc.tile_pool(name="g3", bufs=2))
        g4pool = ctx.enter_context(tc.tile_pool(name="g4", bufs=2))
        tpool = ctx.enter_context(tc.tile_pool(name="t", bufs=3))
        qpool = ctx.enter_context(tc.tile_pool(name="q", bufs=2))
        rpool = ctx.enter_context(tc.tile_pool(name="r", bufs=2))
        hpool = ctx.enter_context(tc.tile_pool(name="hs", bufs=2))
        ypool = ctx.enter_context(tc.tile_pool(name="y", bufs=2))
        # PSUM: full psplit, 7 banks + pu double-buffered -> 8
        pu_pool = ctx.enter_context(tc.tile_pool(name="pu", bufs=2, space="PSUM"))
        pa_pool = ctx.enter_context(tc.tile_pool(name="pa", bufs=1, space="PSUM"))
        pb_pool = ctx.enter_context(tc.tile_pool(name="pb", bufs=1, space="PSUM"))
        pc_pool = ctx.enter_context(tc.tile_pool(name="pc", bufs=1, space="PSUM"))
        p4_pool = ctx.enter_context(tc.tile_pool(name="p4", bufs=1, space="PSUM"))
        py_pool = ctx.enter_context(tc.tile_pool(name="py", bufs=2, space="PSUM"))

        ws_sb = cpool.tile([128, NW, 128], F16)
        nc.sync.dma_start(ws_sb[:], ws_d[:])
        bs_sb = cpool.tile([128, NB], F32)
        nc.sync.dma_start(bs_sb[:], bs_d[:])

        def W(j):
            return ws_sb[:, j, :]

        def bias(j):
            return bs_sb[:, j:j + 1]

        # live per-tile SBUF tiles; pair-wide tensors keyed by tile-pair index
        xs, us, g1s, g2s, g3s, g4s, ts, qs, rs, hss, ys = (
            {} for _ in range(11))
        GRP = 4   # tiles per x-load DMA (2 MB fp16)
        YGRP = 4  # tiles per y-store DMA (1 MB fp16)

        def s0_load(i):  # x prefetch
            if i < GRP:
                # first group: per-tile DMAs so the pipeline starts fast
                xc = x1pool.tile([128, 4, 512], F16, tag="x1", name="x1c")
                nc.sync.dma_start(xc[:], xT_r[:, :, i * 512:(i + 1) * 512])
                xs[i] = (xc, 0)
                return
            if i % GRP:
                return
            xc = xpool.tile([128, 4, GRP * 512], F16, tag="x", name="xc")
            nc.sync.dma_start(xc[:], xT_r[:, :, i * 512:(i + GRP) * 512])
            for j in range(GRP):
                xs[i + j] = (xc, j * 512)

        def s1_u(i):  # u matmuls + copy to SBUF (pair tile, half i%2)
            pu = pu_pool.tile([128, 512], F32, tag="pu", name="pu")
            xc, c0 = xs.pop(i)
            for k in range(4):
                nc.tensor.matmul(pu[:], W(WX0 + k), xc[:, k, c0:c0 + 512],
                                 start=(k == 0), stop=(k == 3))
            if i % 2 == 0:
                us[i // 2] = upool.tile([128, 2, 512], F16, tag="u", name="ut")
            nc.vector.tensor_copy(us[i // 2][:, i % 2, :], pu[:])

        def s2_g1(i):  # pre1 = (Wh+Wu) @ u ; g1 = tanh(+b1)
            pa = pa_pool.tile([128, 512], F32, tag="pa", name="pa")
            nc.tensor.matmul(pa[:], W(SL1), us[i // 2][:, i % 2, :],
                             start=True, stop=True)
            if i % 2 == 0:
                g1s[i // 2] = g1pool.tile([128, 2, 512], F16, tag="g1",
                                          name="g1t")
            nc.scalar.activation(g1s[i // 2][:, i % 2, :], pa[:], AF.Tanh,
                                 bias=bias(0))

        def s3_g2(i):  # pre2 = Wu@u + Wh@g1 ; g2 ; t = g2 - g1
            pb = pb_pool.tile([128, 512], F32, tag="pb", name="pb")
            nc.tensor.matmul(pb[:], W(SLWU), us[i // 2][:, i % 2, :],
                             start=True, stop=False)
            nc.tensor.matmul(pb[:], W(SLWH), g1s[i // 2][:, i % 2, :],
                             start=False, stop=True)
            if i % 2 == 0:
                g2s[i // 2] = g2pool.tile([128, 2, 512], F16, tag="g2",
                                          name="g2t")
            g2h = g2s[i // 2][:, i % 2, :]
            nc.scalar.activation(g2h, pb[:], AF.Tanh, bias=bias(1))
            if i % 2 == 0:
                ts[i // 2] = tpool.tile([128, 2, 512], F16, tag="t", name="tt")
            nc.vector.tensor_sub(ts[i // 2][:, i % 2, :], g2h,
                                 g1s[i // 2][:, i % 2, :])

        def s4_g3(i):  # pre3 = (Wh+Wu)@u + Wh@t ; g3 = tanh(+b1)
            pc = pc_pool.tile([128, 512], F32, tag="pc", name="pc")
            nc.tensor.matmul(pc[:], W(SL1), us[i // 2][:, i % 2, :],
                             start=True, stop=False)
            nc.tensor.matmul(pc[:], W(SLWH), ts[i // 2][:, i % 2, :],
                             start=False, stop=True)
            g3s[i] = g3pool.tile([128, 512], F16, tag="g3", name="g3t")
            nc.scalar.activation(g3s[i][:], pc[:], AF.Tanh, bias=bias(0))

        def s5_g4(i):  # pre4 = (Wu-Wh)@u + 2Wh@g3 - 2Wh@t ; g4 = tanh(+b4)
            p4 = p4_pool.tile([128, 512], F32, tag="p4", name="p4")
            nc.tensor.matmul(p4[:], W(SL4U), us[i // 2][:, i % 2, :],
                             start=True, stop=False)
            nc.tensor.matmul(p4[:], W(SLWH2), g3s.pop(i)[:],
                             start=False, stop=False)
            nc.tensor.matmul(p4[:], W(SLWH2N), ts[i // 2][:, i % 2, :],
                             start=False, stop=True)
            if i % 2 == 0:
                g4s[i // 2] = g4pool.tile([128, 2, 512], F16, tag="g4",
                                          name="g4t")
            nc.scalar.activation(g4s[i // 2][:, i % 2, :], p4[:], AF.Tanh,
                                 bias=bias(2))

        def s6_h(i):  # pair-wide combos on DVE (2x fp16 TT ops):
            # hsum = u - g1 + 2g2 + g4 = (u + g4) + (t + g2)   [t = g2 - g1]
            if i % 2 == 0:
                return
            p = i // 2
            u2 = us.pop(p)
            g12 = g1s.pop(p)  # consumed via t
            g22 = g2s.pop(p)
            g42 = g4s.pop(p)
            t2 = ts.pop(p)
            qs[p] = qpool.tile([128, 2, 512], F16, tag="q", name="qt")
            nc.vector.tensor_add(qs[p][:], u2[:], g42[:])
            rs[p] = rpool.tile([128, 2, 512], F16, tag="r", name="rt")
            nc.vector.tensor_add(rs[p][:], t2[:], g22[:])
            hss[p] = hpool.tile([128, 2, 512], F16, tag="hs", name="hst")
            nc.vector.tensor_add(hss[p][:], qs.pop(p)[:], rs.pop(p)[:])

        def s7_y(i):  # y halves; ACT does out-rows 0-127, DVE does 128-255
            if i % YGRP == 0:
                ys[i // YGRP] = ypool.tile([128, 2, YGRP * 512], F16,
                                           tag="y", name="yt")
            y_sb = ys[i // YGRP]
            c0 = (i % YGRP) * 512
            p, hf = i // 2, i % 2
            hsum = hss[p][:, hf, :]
            pyA = py_pool.tile([128, 512], F32, tag="py", name="pyA")
            nc.tensor.matmul(pyA[:], W(SWO0), hsum, start=True, stop=True)
            nc.scalar.activation(y_sb[:, 0, c0:c0 + 512], pyA[:],
                                 AF.Identity, bias=bias(3))
            pyB = py_pool.tile([128, 512], F32, tag="py", name="pyB")
            nc.tensor.matmul(pyB[:], W(SWO1), hsum, start=True, stop=True)
            nc.vector.tensor_scalar(y_sb[:, 1, c0:c0 + 512], pyB[:],
                                    bias(4), None, ALU.add)
            if hf == 1:
                del hss[p]
            if i % YGRP == YGRP - 1:
                g0 = (i // YGRP) * YGRP
                nc.sync.dma_start(yT_r[:, :, g0 * 512:(g0 + YGRP) * 512], y_sb[:])
                del ys[i // YGRP]

        stages = [s0_load, s6_h, s1_u, s2_g1, s3_g2, s7_y, s4_g3, s5_g4]
        offs = [0, 8, 2, 3, 4, 9, 5, 6]
        for step in range(n_tiles + max(offs)):
            for stage, off in zip(stages, offs):
                i = step - off
                if 0 <= i < n_tiles:
                    stage(i)

    nc.compile()
    return nc


def _get_nc(b_core: int, n_tile: int, variant: str = "v2"):
    key = (b_core, n_tile, variant)
    if key not in _NC_CACHE:
        _NC_CACHE[key] = _build(b_core, n_tile, variant)
    return _NC_CACHE[key]


def _kernel_impl(x, Wx, bx, Wh, Wu, b_ode, Wout, bout,
                 n_cores=N_CORES, n_tile=512, variant='v2', **run_kwargs):
    b = x.shape[0]
    b_core = b // n_cores
    ws, bs = _prep_weights(Wx, bx, Wh, Wu, b_ode, Wout, bout)

    # host-side shard + cast + transpose: [n_cores][D_IN, b_core] fp16
    x = np.asarray(x)
    shards = x.reshape(n_cores, b_core, D_IN).transpose(0, 2, 1)

    nc = _get_nc(b_core, n_tile, variant)
    in_maps = [
        {"xT": np.ascontiguousarray(shards[c], dtype=np.float16),
         "ws": ws, "bs": bs}
        for c in range(n_cores)
    ]
    res = bass_utils.run_bass_kernel_spmd(
        nc, in_maps, core_ids=list(range(n_cores)), **run_kwargs
    )
    y = np.empty((b, D_OUT), dtype=np.float32)
    for c in range(n_cores):
        y[c * b_core:(c + 1) * b_core] = res.results[c]["yT"].T
    return y, res


def kernel(x, Wx, bx, Wh, Wu, b_ode, Wout, bout):
    y, _ = _kernel_impl(x, Wx, bx, Wh, Wu, b_ode, Wout, bout)
    return y
